# revision 30
# baseline (speedup 1.0000x reference)
"""DifferentiableXGB forward on 8 TRN2 NeuronCores.

Data-parallel over batch; per-core batch slice BL=4096.

Layout: batch on PSUM partitions (the x tile is the matmul stationary
operand), tree*leaf (400) on the free axis — the PE array runs at full
128-wide utilization (102400 stream cycles/core vs 131072 for the
t-on-partitions layout) and each LDWEIGHTS hides under the previous
400-cycle stream.

Per batch-group (gsize tiles of 128 rows, psum [128, gsize*512]):
  psum[b, k*100+t] = sum_d x[b,d] W1[t,k,d]      (PE, fp16 in / f32 acc)
  splitb = psum + b1                             (DVE, evacuate to f16)
  leaf   = sigmoid(splitb)                       (ACT)
  s1     = (sum_k splitb) * final_weight[t]      (DVE, pairwise adds, 2x mode)
  prod   = leaf * s1                             (DVE)
  q[b,(bt,k)] = sum_t prod                       (DVE: 2 halvings + short
                                                  1x reduce — reduce has no
                                                  16-bit acceleration)
Final y = q @ fc_w.T + fc_b is 0.26 MFLOP — done on host after gather.
All elementwise stays off GpSimd: its slow 2-input ops on the s-path sat on
the critical path and cost ~16us/rep of wall time.

x is streamed from HBM every rep as two 4MB half-batch transfers that
ping-pong against the other half's compute, so the DMA engines are never
idle; fp16 halves the HBM traffic vs f32. Per-rep wall is PE-bound at
~50-59us depending on the device power state (PE clock 2.0-2.4GHz).
"""
import time
import numpy as np
from contextlib import ExitStack

N_CORES = 8
B, D, T, K = 32768, 1024, 100, 4
BL = B // N_CORES   # batch rows per core
ND = D // 128       # contraction chunks
NBT = BL // 128     # 128-row batch tiles per core (32)
NG = NBT // 4       # groups of 4 batch tiles (8)
C = K * T           # free-axis width per batch tile (400)

_cache = {}


def build(reps=0, mode="full", evac="dve", s_eng="dve", gsize=2, ep_bufs=2, qout="dev"):
    """Build + compile the per-core Bass program. reps>0 executes the
    rep body `reps` times (reps must be odd; hardware loop runs pairs
    with ping-pong x buffers, plus one epilogue body).
    mode: "full" | "nodma" (x loaded once, loop is compute-only) |
    "dmaonly" (loop body is just the x stream-in)."""
    from concourse import bacc
    import concourse.mybir as mybir
    import concourse.tile as tile

    f32, f16 = mybir.dt.float32, mybir.dt.float16
    AF = mybir.ActivationFunctionType
    ALU = mybir.AluOpType
    AX = mybir.AxisListType

    HB = BL // 2  # batch rows per half (2048)
    nc = bacc.Bacc("TRN2", target_bir_lowering=False, debug=False)
    xt = nc.dram_tensor("xt", [2, 128, ND * HB], f16, kind="ExternalInput")
    w = nc.dram_tensor("w", [ND, 128, C], f16, kind="ExternalInput")
    biasb = nc.dram_tensor("biasb", [128, C], f16, kind="ExternalInput")
    fwb = nc.dram_tensor("fwb", [128, T], f16, kind="ExternalInput")
    if qout == "host":
        q = nc.dram_tensor("q", [128, NBT * K * 25], f16, kind="ExternalOutput")
    else:
        q = nc.dram_tensor("q", [128, NBT * K], f32, kind="ExternalOutput")

    with ExitStack() as ctx:
        tc = ctx.enter_context(tile.TileContext(nc))
        cp = ctx.enter_context(tc.tile_pool(name="const", bufs=1))
        xp = ctx.enter_context(tc.tile_pool(name="xp", bufs=1))
        ep = ctx.enter_context(tc.tile_pool(name="ep", bufs=ep_bufs))
        sp = ctx.enter_context(
            tc.tile_pool(name="sp", bufs=8 // gsize, space="PSUM")
        )

        biasb_sb = cp.tile([128, C], f16, name="biasb_sb")
        nc.sync.dma_start(biasb_sb[:], biasb.ap())
        fwb_sb = cp.tile([128, T], f16, name="fwb_sb")
        nc.sync.dma_start(fwb_sb[:], fwb.ap())
        if qout == "host":
            qall = cp.tile([128, NBT * K * 25], f16, name="qall")
        else:
            qall = cp.tile([128, NBT * K], f32, name="qall")
        if mode == "peonly":
            nc.vector.memset(qall[:], 0.0)

        ws = []
        for d in range(ND):
            wt = cp.tile([128, C], f16, name=f"w{d}")
            nc.sync.dma_start(wt[:], w.ap()[d])
            ws.append(wt)

        # one x buffer per batch half; each is a single 4MB DMA and
        # double-buffers against the other half's compute
        xs = [xp.tile([128, ND * HB], f16, name=f"xh{s}") for s in range(2)]
        xap = xt.ap()

        def dma_x(s):
            nc.sync.dma_start(xs[s][:], xap[s])

        G = gsize  # batch tiles per psum group
        bias_bc = biasb_sb[:].unsqueeze(1).broadcast_to([128, G, C])
        fw_bc = fwb_sb[:].unsqueeze(1).broadcast_to([128, G, T])

        def compute(s):
            x_ = xs[s]
            npg = (NBT // 2) // G  # psum groups per half
            for gl in range(npg):
                g = s * npg + gl  # global group index
                psg = sp.tile([128, G * 512], f32, name=f"ps{s}_{gl}", tag="psg")
                for bl in range(G):
                    btl = gl * G + bl  # batch tile within this half
                    for d in range(ND):
                        nc.tensor.matmul(
                            psg[:, bl * 512 : bl * 512 + C],
                            x_[:, d * HB + btl * 128 : d * HB + (btl + 1) * 128],
                            ws[d][:],
                            start=(d == 0),
                            stop=(d == ND - 1),
                        )
                if mode == "peonly":
                    tiny = ep.tile([128, 16], f32, name=f"tn{s}_{g}", tag="tiny")
                    nc.vector.tensor_copy(tiny[:], psg[:, 0:16])
                    continue
                ps4 = psg[:].rearrange("p (bl c) -> p bl c", bl=G, c=512)[
                    :, :, 0:C
                ]
                splitb = ep.tile([128, G * C], f16, name=f"sb{s}_{g}", tag="splitb")
                sb4 = splitb[:].rearrange("p (bl c) -> p bl c", bl=G, c=C)
                if evac == "act":
                    raw = ep.tile([128, G * C], f16, name=f"rw{s}_{g}", tag="raw")
                    rw4 = raw[:].rearrange("p (bl c) -> p bl c", bl=G, c=C)
                    nc.scalar.activation(rw4, ps4, AF.Copy)
                    nc.vector.tensor_add(sb4, rw4, bias_bc)
                else:
                    nc.vector.tensor_add(sb4, ps4, bias_bc)

                leaf = ep.tile([128, G * C], f16, name=f"lf{s}_{g}", tag="leaf")
                nc.scalar.activation(leaf[:], splitb[:], AF.Sigmoid)

                sbk = splitb[:].rearrange(
                    "p (bl k t) -> p bl k t", bl=G, k=K, t=T
                )
                se = nc.gpsimd if s_eng == "gp" else nc.vector
                s2 = ep.tile([128, G * 2 * T], f16, name=f"s2_{s}_{g}", tag="s2")
                s2v = s2[:].rearrange("p (bl k t) -> p bl k t", bl=G, k=2, t=T)
                se.tensor_add(s2v, sbk[:, :, 0:2, :], sbk[:, :, 2:4, :])
                s3 = ep.tile([128, G * T], f16, name=f"s3_{s}_{g}", tag="s3")
                s3v = s3[:].rearrange("p (bl t) -> p bl t", bl=G, t=T)
                se.tensor_add(s3v, s2v[:, :, 0, :], s2v[:, :, 1, :])
                s1 = ep.tile([128, G * T], f16, name=f"s1_{s}_{g}", tag="s1")
                s1v = s1[:].rearrange("p (bl t) -> p bl t", bl=G, t=T)
                se.tensor_mul(s1v, s3v, fw_bc)

                prod = ep.tile([128, G * C], f16, name=f"pr{s}_{g}", tag="prod")
                prv = prod[:].rearrange(
                    "p (bl k t) -> p bl k t", bl=G, k=K, t=T
                )
                lfv = leaf[:].rearrange(
                    "p (bl k t) -> p bl k t", bl=G, k=K, t=T
                )
                s1b = (
                    s1[:]
                    .rearrange("p (bl t) -> p bl t", bl=G, t=T)
                    .unsqueeze(2)
                    .broadcast_to([128, G, K, T])
                )
                nc.vector.tensor_mul(prv, lfv, s1b)

                ph = ep.tile([128, G * K * 50], f16, name=f"ph{s}_{g}", tag="ph")
                phv = ph[:].rearrange(
                    "p (bl k t) -> p bl k t", bl=G, k=K, t=50
                )
                prh = prod[:].rearrange(
                    "p (bl k h t) -> p bl k h t", bl=G, k=K, h=2, t=50
                )
                nc.vector.tensor_add(phv, prh[:, :, :, 0, :], prh[:, :, :, 1, :])

                if qout == "host":
                    p2v = qall[
                        :, g * G * K * 25 : (g + 1) * G * K * 25
                    ].rearrange("p (bl k t) -> p bl k t", bl=G, k=K, t=25)
                else:
                    p2 = ep.tile(
                        [128, G * K * 25], f16, name=f"p2_{s}_{g}", tag="p2"
                    )
                    p2v = p2[:].rearrange(
                        "p (bl k t) -> p bl k t", bl=G, k=K, t=25
                    )
                phh = ph[:].rearrange(
                    "p (bl k h t) -> p bl k h t", bl=G, k=K, h=2, t=25
                )
                nc.vector.tensor_add(p2v, phh[:, :, :, 0, :], phh[:, :, :, 1, :])

                if qout != "host":
                    qv = qall[:, g * G * K : (g + 1) * G * K].rearrange(
                        "p (bl k) -> p bl k", bl=G, k=K
                    )
                    nc.vector.tensor_reduce(qv, p2v, axis=AX.X, op=ALU.add)

        dma_x(0)
        if mode == "nodma":
            dma_x(1)
            if reps > 1:
                with tc.For_i(0, reps - 1, 1):
                    compute(0)
                    compute(1)
            compute(0)
            compute(1)
        elif mode == "dmaonly":
            if reps > 1:
                with tc.For_i(0, reps - 1, 1):
                    dma_x(1)
                    dma_x(0)
            dma_x(1)
            compute(0)
            compute(1)
        else:
            # one rep per body; x for the other half streams in while this
            # half computes, so the DMA engines are never idle
            if reps > 1:
                with tc.For_i(0, reps, 1):
                    dma_x(1)
                    compute(0)
                    dma_x(0)
                    compute(1)
            else:
                dma_x(1)
                compute(0)
                compute(1)

        nc.sync.dma_start(q.ap(), qall[:])
    nc.compile()
    return nc


def make_in_maps(x, W1, b1, final_weight, fc_w, fc_b):
    x = np.asarray(x, np.float32)
    W1 = np.asarray(W1, np.float32)
    b1 = np.asarray(b1, np.float32)
    final_weight = np.asarray(final_weight, np.float32)

    w8 = np.ascontiguousarray(
        W1.transpose(2, 1, 0).reshape(ND, 128, C).astype(np.float16)
    )
    biasb = np.ascontiguousarray(
        np.broadcast_to(b1.T.reshape(1, C), (128, C)).astype(np.float16)
    )
    fwb = np.ascontiguousarray(
        np.broadcast_to(
            final_weight.reshape(1, T).astype(np.float16), (128, T)
        )
    )

    HB = BL // 2
    in_maps = []
    for c in range(N_CORES):
        xc = x[c * BL : (c + 1) * BL, :].T.astype(np.float16)  # [D, BL]
        xc3 = xc.reshape(ND, 128, BL)
        xt = np.stack(
            [
                np.ascontiguousarray(
                    xc3[:, :, h * HB : (h + 1) * HB].transpose(1, 0, 2)
                ).reshape(128, ND * HB)
                for h in range(2)
            ]
        )
        in_maps.append(
            {
                "xt": np.ascontiguousarray(xt),
                "w": w8,
                "biasb": biasb,
                "fwb": fwb,
            }
        )
    return in_maps


def finish(q_per_core, fc_w, fc_b):
    """Host epilogue: q [128, 32*4] (or partial sums [128, 32*4*25])
    per core -> y [B, 2]."""
    fc_w = np.asarray(fc_w, np.float32)
    fc_b = np.asarray(fc_b, np.float32)
    out = np.empty((B, 2), np.float32)
    for c in range(N_CORES):
        qc = np.asarray(q_per_core[c], np.float32)
        if qc.size == 128 * NBT * K * 25:
            qc = qc.reshape(128, NBT, K, 25).sum(-1)
        qc = qc.reshape(128, NBT, K)
        yc = np.einsum("pbk,jk->bpj", qc, fc_w).reshape(BL, 2) + fc_b
        out[c * BL : (c + 1) * BL] = yc
    return out


def kernel(x, W1, b1, final_weight, fc_w, fc_b):
    from concourse.bass_utils import run_bass_kernel_spmd

    if "nc" not in _cache:
        _cache["nc"] = build()
    nc = _cache["nc"]
    in_maps = make_in_maps(x, W1, b1, final_weight, fc_w, fc_b)

    last_err = None
    for attempt in range(3):
        try:
            res = run_bass_kernel_spmd(nc, in_maps, core_ids=list(range(N_CORES)))
            break
        except Exception as e:  # transient device wedge: wait for recovery
            last_err = e
            time.sleep(90)
    else:
        raise last_err

    return finish([res.results[c]["q"] for c in range(N_CORES)], fc_w, fc_b)
